# revision 23
# baseline (speedup 1.0000x reference)
"""Trainium2 Bass kernel for nn_KGPathReasoner.

8-core SPMD, data-parallel over the entity-pair dimension P.
Each core handles 256 pairs = 2560 paths; embedding tables + weights replicated.

Device layout is feature-major (features on SBUF partitions, tokens on the free
dim) throughout:
  - tokens are sorted by path length (descending) on host, so LSTM step t only
    runs over the chunks that still contain live paths; the per-step chunk
    counts are computed from the actual path_lens and baked into the program
    (programs are cached per chunk-count signature),
  - embedding tables are pre-cast to bf16 on host; rows are gathered
    token-major via indirect DMA and transposed feature-major on the tensor
    engine (single-pass bf16 matmul against identity),
  - all matmul operands are bf16 (PSUM accumulation stays fp32); weights are
    pre-folded on host:
      M_cat = [kg_proj_w.T @ w_ih.T[:512] ; kg_proj_w.T @ w_ih.T[512:]]
    so the kg projection never materializes,
  - the LSTM runs over 512-token chunks with gates accumulated in PSUM
    (x-side + h-side matmuls), sigmoid/tanh on the scalar engine with the fused
    per-partition bias, cell updates on the vector engine (c stays fp32, h is
    written bf16 for the next step's matmul),
  - h(len-1) selection is a predicated copy against masks (len == t+1); the
    selected embeddings are transposed token-major, round-tripped through a
    DRAM scratch buffer, and gathered back in the original token order (the
    scratch output buffer is zero-donated, so len-0 paths read back as zero),
  - attention uses a block-diagonal trick: groups of 8 pairs = 80 tokens, all
    4 heads' [80x80] scores packed into one PSUM bank, exp on ACT, block-diag
    mask multiply, column-sum via ones-matmul, reciprocal + partition
    broadcast, and a v.T @ attn matmul giving ctx feature-major directly,
  - attn_out + mean-over-paths + path_proj fold into a single matmul
    (attention rows sum to 1, everything after ctx is linear):
      out = Wf @ sum_k(ctx') + bf,  Wf = wpp @ wao / K,
      bf = wpp @ wao @ bv + wpp @ bao + bpp.
"""

import numpy as np
import ml_dtypes

BF16 = ml_dtypes.bfloat16

NCORES = 8
P, KP, L = 2048, 10, 3
E, H = 256, 512
N_ENT, N_REL = 10000, 200
NHEADS, DH = 4, 128
P_LOC = P // NCORES           # 256 pairs per core
N_LOC = P_LOC * KP            # 2560 paths per core
CH = 512                      # LSTM token chunk
NCH = N_LOC // CH             # 5
NG = CH // 128                # 4 gather groups of 128 per chunk
NTG = N_LOC // 128            # 20 token groups of 128
AG = 80                       # attention group = 8 pairs * 10 paths
PAIRS_G = AG // KP            # 8
AOCH = 320                    # attn chunk (32 pairs)
NAOCH = N_LOC // AOCH         # 8

_PROGS = {}


def _build_program(c_steps):
    import concourse.bass as bass
    import concourse.mybir as mybir
    import concourse.tile as tile
    from concourse import bacc

    f32 = mybir.dt.float32
    bf16 = mybir.dt.bfloat16
    i32 = mybir.dt.int32
    AF = mybir.ActivationFunctionType
    OP = mybir.AluOpType

    NJ = sum(c_steps) * NG            # gather-index columns
    n_proc = sum(1 for s in c_steps if s > 0)

    nc = bacc.Bacc()

    # ---- DRAM parameters (per core) ----
    ent_table = nc.declare_dram_parameter("ent_table_bf", [N_ENT, E], bf16, isOutput=False)
    rel_table = nc.declare_dram_parameter("rel_table_bf", [N_REL, E], bf16, isOutput=False)
    rel_idx_d = nc.declare_dram_parameter("rel_idx_p", [128, NJ], i32, isOutput=False)
    ent_idx_d = nc.declare_dram_parameter("ent_idx_p", [128, NJ], i32, isOutput=False)
    gb_d = nc.declare_dram_parameter("gb_idx", [128, NTG], i32, isOutput=False)
    lens_d = nc.declare_dram_parameter("lens_row", [1, N_LOC], f32, isOutput=False)
    mcat_d = nc.declare_dram_parameter("mcat_t", [2 * E, 4 * H], bf16, isOutput=False)
    whh_d = nc.declare_dram_parameter("whh_t", [H, 4 * H], bf16, isOutput=False)
    gbias_d = nc.declare_dram_parameter("gate_bias", [128, 16], f32, isOutput=False)
    wq_d = nc.declare_dram_parameter("wq_t", [H, H], bf16, isOutput=False)
    wk_d = nc.declare_dram_parameter("wk_t", [H, H], bf16, isOutput=False)
    wv_d = nc.declare_dram_parameter("wv_t", [H, H], bf16, isOutput=False)
    bq_d = nc.declare_dram_parameter("bq_p", [128, 4], f32, isOutput=False)
    bk_d = nc.declare_dram_parameter("bk_p", [128, 4], f32, isOutput=False)
    # fused tail: out = Wf @ sum_k(ctx) + bf, Wf = wpp @ wao / K,
    # bf = wpp @ wao @ bv + wpp @ bao + bpp (attention rows sum to 1)
    wf_d = nc.declare_dram_parameter("wf_t", [H, H], bf16, isOutput=False)
    bf_d = nc.declare_dram_parameter("bf_p", [128, 4], f32, isOutput=False)
    bdm_d = nc.declare_dram_parameter("bd_mask", [128, NHEADS * AG], bf16, isOutput=False)
    # token-major path-embedding scratch (sorted order); declared as an output
    # so PJRT donates a zeroed buffer -> unwritten (len-0 / unprocessed) rows
    # read back as zero
    hscr_d = nc.declare_dram_parameter("h_scr", [N_LOC, H], bf16, isOutput=True)
    out_d = nc.declare_dram_parameter("out", [H, P_LOC], f32, isOutput=True)

    with tile.TileContext(nc) as tc:
        # ---------- persistent pool (spans both phases) ----------
        with tc.tile_pool(name="persist", bufs=1) as pp:
            # h_sel: selected h, feature-major, SORTED token order
            h_sel = [pp.tile([128, n_proc * CH], bf16, name=f"h_sel{i}") for i in range(4)]
            for hs in h_sel:
                nc.gpsimd.memset(hs[:], 0.0)
            # h_org: path_emb, feature-major, ORIGINAL token order
            h_org = [pp.tile([128, N_LOC], bf16, name=f"h_org{i}") for i in range(4)]

            ones_t = pp.tile([128, H], bf16, name="ones_t")
            nc.vector.memset(ones_t[:], 1.0)

            ident = pp.tile([128, 128], bf16, name="ident")
            from concourse.masks import make_identity
            make_identity(nc, ident[:])

            bdm_sb = pp.tile([128, NHEADS * AG], bf16, name="bdm_sb")
            nc.sync.dma_start(out=bdm_sb[:], in_=bdm_d[:, :])

            # ---------- phase 1: encode + LSTM (sorted order) ----------
            with tc.tile_pool(name="lw", bufs=1) as lw, \
                 tc.tile_pool(name="lstm_sb", bufs=2) as ls, \
                 tc.tile_pool(name="gath", bufs=16) as gp, \
                 tc.tile_pool(name="xcat", bufs=8) as xp, \
                 tc.tile_pool(name="sig", bufs=8) as sg, \
                 tc.tile_pool(name="mb", bufs=3) as mbp, \
                 tc.tile_pool(name="xtp", bufs=2, space="PSUM") as xtp, \
                 tc.tile_pool(name="gpsum", bufs=4, space="PSUM") as gpsum:

                mcat_sb = [lw.tile([128, 4 * H], bf16, name=f"mcat{i}") for i in range(4)]
                whh_sb = [lw.tile([128, 4 * H], bf16, name=f"whh{i}") for i in range(4)]
                for i in range(4):
                    nc.sync.dma_start(out=mcat_sb[i][:], in_=mcat_d[i * 128:(i + 1) * 128, :])
                    nc.sync.dma_start(out=whh_sb[i][:], in_=whh_d[i * 128:(i + 1) * 128, :])
                gb_sb = lw.tile([128, NTG], i32, name="gb_sb")
                nc.sync.dma_start(out=gb_sb[:], in_=gb_d[:, :])
                gb_psb = lw.tile([128, 16], f32, name="gb_psb")
                nc.sync.dma_start(out=gb_psb[:], in_=gbias_d[:, :])
                ridx_sb = lw.tile([128, NJ], i32, name="ridx_sb")
                eidx_sb = lw.tile([128, NJ], i32, name="eidx_sb")
                nc.sync.dma_start(out=ridx_sb[:], in_=rel_idx_d[:, :])
                nc.sync.dma_start(out=eidx_sb[:], in_=ent_idx_d[:, :])
                # lens broadcast across partitions once; per-chunk masks via is_equal
                lens_sb = lw.tile([1, N_LOC], f32, name="lens_sb")
                nc.sync.dma_start(out=lens_sb[:], in_=lens_d[:, :])
                lens_b = lw.tile([128, N_LOC], f32, name="lens_b")
                nc.gpsimd.partition_broadcast(lens_b[:], lens_sb[:], channels=128)

                jctr = 0
                for c in range(NCH):
                    h_prev = [None] * 4
                    c_prev = [None] * 4
                    for t in range(c_steps[c]):
                        # mask (lens == t+1), [128, CH] from broadcast lens
                        mb = mbp.tile([128, CH], i32, name="mb", tag="mb")
                        nc.vector.tensor_scalar(
                            out=mb[:], in0=lens_b[:, c * CH:(c + 1) * CH],
                            scalar1=float(t + 1), scalar2=None, op0=OP.is_equal)

                        # gathers (token-major [128, 256] bf16 per 128-token group)
                        gts = []
                        for g in range(NG):
                            j = jctr + g
                            grel = gp.tile([128, E], bf16, name="grel", tag="gath")
                            gent = gp.tile([128, E], bf16, name="gent", tag="gath")
                            nc.gpsimd.indirect_dma_start(
                                out=grel[:], out_offset=None, in_=rel_table[:, :],
                                in_offset=bass.IndirectOffsetOnAxis(
                                    ap=ridx_sb[:, j:j + 1], axis=0))
                            nc.gpsimd.indirect_dma_start(
                                out=gent[:], out_offset=None, in_=ent_table[:, :],
                                in_offset=bass.IndirectOffsetOnAxis(
                                    ap=eidx_sb[:, j:j + 1], axis=0))
                            gts.append((grel, gent))
                        jctr += NG

                        # transpose to feature-major xc [4][128, CH] on PE
                        # (plain matmul against identity; is_transpose=True
                        # would put both sem waits on the LW struct, which
                        # codegen rejects)
                        xt_rel = xtp.tile([128, 2 * CH], f32, name="xt_rel", tag="xt", space="PSUM")
                        xt_ent = xtp.tile([128, 2 * CH], f32, name="xt_ent", tag="xt", space="PSUM")
                        for g in range(NG):
                            grel, gent = gts[g]
                            for half in range(2):
                                nc.tensor.matmul(
                                    out=xt_rel[:, half * CH + g * 128:half * CH + g * 128 + 128],
                                    lhsT=grel[:, half * 128:(half + 1) * 128],
                                    rhs=ident[:], start=True, stop=True)
                                nc.tensor.matmul(
                                    out=xt_ent[:, half * CH + g * 128:half * CH + g * 128 + 128],
                                    lhsT=gent[:, half * 128:(half + 1) * 128],
                                    rhs=ident[:], start=True, stop=True)
                        xc = []
                        for i in range(4):
                            xi = xp.tile([128, CH], bf16, name="xi", tag="xcat")
                            src = (xt_rel, xt_ent)[i // 2]
                            nc.vector.tensor_copy(out=xi[:], in_=src[:, (i % 2) * CH:(i % 2 + 1) * CH])
                            xc.append(xi)

                        # gates: one PSUM bank per gate [i, f, g, o]
                        h_new = [None] * 4
                        c_new = [None] * 4
                        for ft in range(4):
                            gps = {}
                            for gi, m in enumerate((ft, 4 + ft, 8 + ft, 12 + ft)):
                                if t == 0 and gi == 1:
                                    continue  # forget gate unused when c==0
                                gt = gpsum.tile([128, CH], f32, name="gt", tag="gpsum", space="PSUM")
                                gps[gi] = gt
                                for kt in range(4):
                                    nc.tensor.matmul(
                                        out=gt[:], lhsT=mcat_sb[kt][:, m * 128:(m + 1) * 128],
                                        rhs=xc[kt], start=(kt == 0), stop=(t == 0 and kt == 3))
                                if t > 0:
                                    for kt in range(4):
                                        nc.tensor.matmul(
                                            out=gt[:], lhsT=whh_sb[kt][:, m * 128:(m + 1) * 128],
                                            rhs=h_prev[kt], start=False, stop=(kt == 3))
                            si = sg.tile([128, CH], f32, name="si", tag="sig")
                            tg = sg.tile([128, CH], f32, name="tg", tag="sig")
                            so = sg.tile([128, CH], f32, name="so", tag="sig")
                            nc.scalar.activation(out=si[:], in_=gps[0][:],
                                                 func=AF.Sigmoid, bias=gb_psb[:, ft:ft + 1])
                            nc.scalar.activation(out=tg[:], in_=gps[2][:],
                                                 func=AF.Tanh, bias=gb_psb[:, 8 + ft:9 + ft])
                            nc.scalar.activation(out=so[:], in_=gps[3][:],
                                                 func=AF.Sigmoid, bias=gb_psb[:, 12 + ft:13 + ft])
                            cn = ls.tile([128, CH], f32, name="cn", tag=f"c{ft}", bufs=2)
                            if t == 0:
                                nc.vector.tensor_tensor(out=cn[:], in0=si[:], in1=tg[:], op=OP.mult)
                            else:
                                sf = sg.tile([128, CH], f32, name="sf", tag="sig")
                                nc.scalar.activation(out=sf[:], in_=gps[1][:],
                                                     func=AF.Sigmoid, bias=gb_psb[:, 4 + ft:5 + ft])
                                tmp = sg.tile([128, CH], f32, name="tmp", tag="sig")
                                nc.vector.tensor_tensor(out=cn[:], in0=sf[:], in1=c_prev[ft][:], op=OP.mult)
                                nc.vector.tensor_tensor(out=tmp[:], in0=si[:], in1=tg[:], op=OP.mult)
                                nc.vector.tensor_tensor(out=cn[:], in0=cn[:], in1=tmp[:], op=OP.add)
                            tc_t = sg.tile([128, CH], f32, name="tc_t", tag="sig")
                            nc.scalar.activation(out=tc_t[:], in_=cn[:], func=AF.Tanh)
                            hn = ls.tile([128, CH], bf16, name="hn", tag=f"h{ft}", bufs=2)
                            nc.vector.tensor_tensor(out=hn[:], in0=so[:], in1=tc_t[:], op=OP.mult)
                            nc.vector.copy_predicated(
                                out=h_sel[ft][:, c * CH:(c + 1) * CH], mask=mb[:], data=hn[:])
                            h_new[ft] = hn
                            c_new[ft] = cn
                        h_prev = h_new
                        c_prev = c_new

                # ---- phase 1.5: unsort path embeddings via DRAM round-trip ----
                # sorted h_sel -> token-major h_scr rows (PE transpose + DMA out)
                wr_insts = []
                for c in range(n_proc):
                    for g in range(NG):
                        s0 = c * CH + g * 128
                        tp = gpsum.tile([128, CH], f32, name="tp", tag="gpsum", space="PSUM")
                        for ft in range(4):
                            nc.tensor.matmul(
                                out=tp[:, ft * 128:(ft + 1) * 128],
                                lhsT=h_sel[ft][:, s0:s0 + 128],
                                rhs=ident[:], start=True, stop=True)
                        htm = xp.tile([128, CH], bf16, name="htm", tag="xcat")
                        nc.vector.tensor_copy(out=htm[:], in_=tp[:])
                        wr = nc.sync.dma_start(out=hscr_d[s0:s0 + 128, :], in_=htm[:])
                        wr_insts.append(wr)

                # gather back in ORIGINAL token order + transpose feature-major
                for g2 in range(NTG):
                    hg = gp.tile([128, H], bf16, name="hg", tag="gath")
                    rd = nc.gpsimd.indirect_dma_start(
                        out=hg[:], out_offset=None, in_=hscr_d[:, :],
                        in_offset=bass.IndirectOffsetOnAxis(
                            ap=gb_sb[:, g2:g2 + 1], axis=0))
                    for wr in wr_insts:
                        tile.add_dep_helper(rd.ins, wr.ins, reason="h_scr RAW round-trip")
                    tp2 = gpsum.tile([128, CH], f32, name="tp2", tag="gpsum", space="PSUM")
                    for ft in range(4):
                        nc.tensor.matmul(
                            out=tp2[:, ft * 128:(ft + 1) * 128],
                            lhsT=hg[:, ft * 128:(ft + 1) * 128],
                            rhs=ident[:], start=True, stop=True)
                    for ft in range(4):
                        nc.vector.tensor_copy(
                            out=h_org[ft][:, g2 * 128:(g2 + 1) * 128],
                            in_=tp2[:, ft * 128:(ft + 1) * 128])

            # ---------- phase 2: attention (original order, dense) ----------
            NGG = N_LOC // AG  # 32 independent pair-groups
            with tc.tile_pool(name="aw", bufs=1) as aw, \
                 tc.tile_pool(name="asml", bufs=8) as asml, \
                 tc.tile_pool(name="aps2", bufs=2, space="PSUM") as aps2, \
                 tc.tile_pool(name="aps1", bufs=2, space="PSUM") as aps1, \
                 tc.tile_pool(name="aps3", bufs=2, space="PSUM") as aps3, \
                 tc.tile_pool(name="aps4", bufs=2, space="PSUM") as aps4:  # noqa

                wq_sb = [aw.tile([128, H], bf16, name=f"wq{i}") for i in range(4)]
                wk_sb = [aw.tile([128, H], bf16, name=f"wk{i}") for i in range(4)]
                wv_sb = [aw.tile([128, H], bf16, name=f"wv{i}") for i in range(4)]
                wf_sb = [aw.tile([128, H], bf16, name=f"wf{i}") for i in range(4)]
                for i in range(4):
                    nc.sync.dma_start(out=wq_sb[i][:], in_=wq_d[i * 128:(i + 1) * 128, :])
                    nc.sync.dma_start(out=wk_sb[i][:], in_=wk_d[i * 128:(i + 1) * 128, :])
                    nc.sync.dma_start(out=wv_sb[i][:], in_=wv_d[i * 128:(i + 1) * 128, :])
                    nc.sync.dma_start(out=wf_sb[i][:], in_=wf_d[i * 128:(i + 1) * 128, :])
                bq_sb = aw.tile([128, 4], f32, name="bq_sb")
                bk_sb = aw.tile([128, 4], f32, name="bk_sb")
                bf_sb = aw.tile([128, 4], f32, name="bf_sb")
                nc.sync.dma_start(out=bq_sb[:], in_=bq_d[:, :])
                nc.sync.dma_start(out=bk_sb[:], in_=bk_d[:, :])
                nc.sync.dma_start(out=bf_sb[:], in_=bf_d[:, :])

                agg_sb = [aw.tile([128, P_LOC], f32, name=f"agg{i}") for i in range(4)]
                agg_bf = [aw.tile([128, P_LOC], bf16, name=f"aggb{i}") for i in range(4)]

                # A. dense q/k for all tokens (bias added via ACT copy)
                q_all = [aw.tile([128, N_LOC], bf16, name=f"q_all{i}") for i in range(4)]
                k_all = [aw.tile([128, N_LOC], bf16, name=f"k_all{i}") for i in range(4)]
                for m in range(4):
                    for cc in range(NCH):
                        ss = cc * CH
                        qps = aps2.tile([128, CH], f32, name="qps", tag="aps2", space="PSUM")
                        kps = aps2.tile([128, CH], f32, name="kps", tag="aps2", space="PSUM")
                        for kt in range(4):
                            nc.tensor.matmul(
                                out=qps[:], lhsT=wq_sb[kt][:, m * 128:(m + 1) * 128],
                                rhs=h_org[kt][:, ss:ss + CH], start=(kt == 0), stop=(kt == 3))
                            nc.tensor.matmul(
                                out=kps[:], lhsT=wk_sb[kt][:, m * 128:(m + 1) * 128],
                                rhs=h_org[kt][:, ss:ss + CH], start=(kt == 0), stop=(kt == 3))
                        nc.scalar.activation(out=q_all[m][:, ss:ss + CH], in_=qps[:],
                                             func=AF.Identity, bias=bq_sb[:, m:m + 1])
                        nc.scalar.activation(out=k_all[m][:, ss:ss + CH], in_=kps[:],
                                             func=AF.Identity, bias=bk_sb[:, m:m + 1])

                # B. v token-major for all 32 groups (bv folded into bf)
                v_tm = []
                for gg in range(NGG):
                    so_ = gg * AG
                    vp = aps1.tile([128, H], f32, name="vp", tag="aps1", space="PSUM")
                    for kt in range(4):
                        nc.tensor.matmul(
                            out=vp[:AG, :], lhsT=h_org[kt][:, so_:so_ + AG],
                            rhs=wv_sb[kt][:], start=(kt == 0), stop=(kt == 3))
                    vsb = aw.tile([128, H], bf16, name=f"vsb{gg}")
                    nc.scalar.activation(out=vsb[:AG, :], in_=vp[:AG, :], func=AF.Copy)
                    v_tm.append(vsb)

                # C. attention core: 32 independent group chains
                ctx_all = [aw.tile([128, N_LOC], bf16, name=f"ctx{i}") for i in range(4)]
                for gg in range(NGG):
                    o = gg * AG
                    exb = asml.tile([128, NHEADS * AG], bf16, name="exb", tag="exb")
                    # all 4 heads' scores in one PSUM bank (start only on the
                    # first head: flags=0 writes overwrite untouched columns)
                    sc = aps3.tile([128, NHEADS * AG], f32, name="sc", tag="aps3", space="PSUM")
                    for hh in range(NHEADS):
                        nc.tensor.matmul(
                            out=sc[:AG, hh * AG:(hh + 1) * AG], lhsT=k_all[hh][:, o:o + AG],
                            rhs=q_all[hh][:, o:o + AG], start=(hh == 0), stop=(hh == 3))
                    exu = asml.tile([128, NHEADS * AG], bf16, name="exu", tag="exu")
                    nc.scalar.activation(out=exu[:AG, :], in_=sc[:AG, :],
                                         func=AF.Exp, scale=float(1.0 / np.sqrt(DH)))
                    nc.vector.tensor_tensor(
                        out=exb[:AG, :], in0=exu[:AG, :],
                        in1=bdm_sb[:AG, :], op=OP.mult)
                    cs = aps3.tile([1, NHEADS * AG], f32, name="cs", tag="aps3", space="PSUM")
                    nc.tensor.matmul(out=cs[:1, :], lhsT=ones_t[:AG, :1],
                                     rhs=exb[:AG, :], start=True, stop=True)
                    css = asml.tile([1, NHEADS * AG], f32, name="css", tag="css")
                    nc.vector.reciprocal(out=css[:], in_=cs[:1, :])
                    rb = asml.tile([128, NHEADS * AG], f32, name="rb", tag="rb")
                    nc.gpsimd.partition_broadcast(rb[:], css[:], channels=128)
                    for hh in range(NHEADS):
                        cxp = aps4.tile([128, AG], f32, name="cxp", tag="aps4", space="PSUM")
                        nc.tensor.matmul(
                            out=cxp[:, :], lhsT=v_tm[gg][:AG, hh * 128:(hh + 1) * 128],
                            rhs=exb[:AG, hh * AG:(hh + 1) * AG], start=True, stop=True)
                        nc.vector.scalar_tensor_tensor(
                            out=ctx_all[hh][:, o:o + AG], in0=cxp[:, :],
                            scalar=1.0, in1=rb[:, hh * AG:(hh + 1) * AG],
                            op0=OP.mult, op1=OP.mult)

                # D. sum ctx over each pair's K paths (tail matmul is folded)
                for m in range(4):
                    nc.vector.reduce_sum(
                        out=agg_sb[m][:],
                        in_=ctx_all[m][:].rearrange("p (a k) -> p a k", k=KP),
                        axis=mybir.AxisListType.X)
                    nc.vector.tensor_copy(out=agg_bf[m][:], in_=agg_sb[m][:])

                # fused attn_out + mean + path_proj: out = Wf @ aggS + bf
                for m in range(4):
                    pps = aps4.tile([128, P_LOC], f32, name="pps", tag="aps4", space="PSUM")
                    for kt in range(4):
                        nc.tensor.matmul(
                            out=pps[:], lhsT=wf_sb[kt][:, m * 128:(m + 1) * 128],
                            rhs=agg_bf[kt][:], start=(kt == 0), stop=(kt == 3))
                    osb = asml.tile([128, P_LOC], f32, name="osb", tag="osb")
                    nc.vector.tensor_scalar_add(out=osb[:], in0=pps[:], scalar1=bf_sb[:, m:m + 1])
                    nc.sync.dma_start(out=out_d[m * 128:(m + 1) * 128, :], in_=osb[:])

    nc.compile()
    return nc


def _chunk_steps(path_lens):
    """Steps needed per 512-token chunk (max over cores), tokens sorted by
    length descending within each core."""
    cs = [0] * NCH
    for core in range(NCORES):
        lens = path_lens[core * P_LOC:(core + 1) * P_LOC].reshape(-1)
        for t in range(L):
            n = int((lens >= t + 1).sum())
            k = -(-n // CH)  # ceil
            for c in range(k):
                cs[c] = max(cs[c], t + 1)
    return tuple(cs)


def _prep_host(inputs, c_steps):
    """Fold weights and lay out indices host-side. Returns (shared, per_core)."""
    f = np.float32
    kg_proj_w = np.asarray(inputs["kg_proj_w"], f)      # [H, E]
    kg_proj_b = np.asarray(inputs["kg_proj_b"], f)      # [H]
    w_ih = np.asarray(inputs["w_ih"], f)                # [4H, 2H]
    w_hh = np.asarray(inputs["w_hh"], f)                # [4H, H]
    b_ih = np.asarray(inputs["b_ih"], f)
    b_hh = np.asarray(inputs["b_hh"], f)
    attn_in_w = np.asarray(inputs["attn_in_w"], f)      # [3H, H]
    attn_in_b = np.asarray(inputs["attn_in_b"], f)
    attn_out_w = np.asarray(inputs["attn_out_w"], f)    # [H, H]
    attn_out_b = np.asarray(inputs["attn_out_b"], f)
    path_proj_w = np.asarray(inputs["path_proj_w"], f)  # [H, H]
    path_proj_b = np.asarray(inputs["path_proj_b"], f)

    W1 = w_ih[:, :H].T                                   # [H, 4H] (rel_p part)
    W2 = w_ih[:, H:].T                                   # [H, 4H] (ent_p part)
    M_r = kg_proj_w.T @ W1                               # [E, 4H]
    M_e = kg_proj_w.T @ W2                               # [E, 4H]
    mcat_t = np.ascontiguousarray(np.concatenate([M_r, M_e], axis=0))  # [2E, 4H]
    gate_bias = kg_proj_b @ W1 + kg_proj_b @ W2 + b_ih + b_hh          # [4H]

    bd = np.zeros((128, AG), f)
    for pg in range(PAIRS_G):
        bd[pg * KP:(pg + 1) * KP, pg * KP:(pg + 1) * KP] = 1.0

    # fused tail: out = Wf @ sum_k(ctx) + bf  (attention rows sum to 1, so bv
    # passes through; mean + attn_out + path_proj are all linear)
    bv = attn_in_b[2 * H:]
    ppw_aow = path_proj_w @ attn_out_w                   # [H, H]
    wf_eff = ppw_aow / KP
    bf_vec = ppw_aow @ bv + path_proj_w @ attn_out_b + path_proj_b

    shared = {
        "ent_table_bf": np.ascontiguousarray(np.asarray(inputs["ent_table"], f).astype(BF16)),
        "rel_table_bf": np.ascontiguousarray(np.asarray(inputs["rel_table"], f).astype(BF16)),
        "mcat_t": np.ascontiguousarray(mcat_t.astype(BF16)),
        "whh_t": np.ascontiguousarray(w_hh.T.astype(BF16)),
        "gate_bias": np.ascontiguousarray(gate_bias.reshape(16, 128).T),
        "wq_t": np.ascontiguousarray(attn_in_w[:H].T.astype(BF16)),
        "wk_t": np.ascontiguousarray(attn_in_w[H:2 * H].T.astype(BF16)),
        "wv_t": np.ascontiguousarray(attn_in_w[2 * H:].T.astype(BF16)),
        "bq_p": np.ascontiguousarray(attn_in_b[:H].reshape(4, 128).T),
        "bk_p": np.ascontiguousarray(attn_in_b[H:2 * H].reshape(4, 128).T),
        "wf_t": np.ascontiguousarray(wf_eff.T.astype(BF16)),
        "bf_p": np.ascontiguousarray(bf_vec.reshape(4, 128).T),
        "bd_mask": np.ascontiguousarray(np.tile(bd, (1, NHEADS)).astype(BF16)),
    }

    rel_idx = np.asarray(inputs["rel_idx"])              # [P, K, L] int32
    ent_idx = np.asarray(inputs["ent_idx"])
    path_lens = np.asarray(inputs["path_lens"])          # [P, K] int32

    NJ = sum(c_steps) * NG
    per_core = []
    for core in range(NCORES):
        sl = slice(core * P_LOC, (core + 1) * P_LOC)
        lens = path_lens[sl].reshape(N_LOC)
        perm = np.argsort(-lens, kind="stable")
        inv = np.empty(N_LOC, np.int64)
        inv[perm] = np.arange(N_LOC)
        ri = rel_idx[sl].reshape(N_LOC, L)[perm]
        ei = ent_idx[sl].reshape(N_LOC, L)[perm]
        rj = np.empty((128, NJ), np.int32)
        ej = np.empty((128, NJ), np.int32)
        jj = 0
        for c in range(NCH):
            for t in range(c_steps[c]):
                for g in range(NG):
                    s0 = c * CH + g * 128
                    rj[:, jj] = ri[s0:s0 + 128, t]
                    ej[:, jj] = ei[s0:s0 + 128, t]
                    jj += 1
        lens_row = lens[perm].reshape(1, N_LOC).astype(f)
        gb = np.ascontiguousarray(inv.reshape(NTG, 128).T.astype(np.int32))
        per_core.append({"rel_idx_p": rj, "ent_idx_p": ej, "lens_row": lens_row,
                         "gb_idx": gb})
    return shared, per_core


def _run(inputs, trace=False):
    from concourse.bass_utils import run_bass_kernel_spmd
    c_steps = _chunk_steps(np.asarray(inputs["path_lens"]))
    if c_steps not in _PROGS:
        _PROGS[c_steps] = _build_program(c_steps)
    prog = _PROGS[c_steps]
    shared, per_core = _prep_host(inputs, c_steps)
    in_maps = [{**shared, **pc} for pc in per_core]
    res = run_bass_kernel_spmd(prog, in_maps, list(range(NCORES)), trace=trace)
    out = np.concatenate([np.ascontiguousarray(r["out"].T) for r in res.results], axis=0)
    return out, res


def kernel(**inputs):
    out, _ = _run(inputs, trace=False)
    return out


# revision 29
# speedup vs baseline: 1.0402x; 1.0402x over previous
"""Trainium2 Bass kernel for nn_KGPathReasoner.

8-core SPMD, data-parallel over the entity-pair dimension P.
Each core handles 256 pairs = 2560 paths; embedding tables + weights replicated.

Device layout is feature-major (features on SBUF partitions, tokens on the free
dim) throughout:
  - tokens are sorted by path length (descending) on host, so LSTM step t only
    runs over the chunks that still contain live paths; the per-step chunk
    counts are computed from the actual path_lens and baked into the program
    (programs are cached per chunk-count signature),
  - embedding tables are pre-cast to bf16 on host; rows are gathered
    token-major via indirect DMA and transposed feature-major on the tensor
    engine (single-pass bf16 matmul against identity),
  - all matmul operands are bf16 (PSUM accumulation stays fp32); weights are
    pre-folded on host:
      M_cat = [kg_proj_w.T @ w_ih.T[:512] ; kg_proj_w.T @ w_ih.T[512:]]
    so the kg projection never materializes,
  - the LSTM runs over 512-token chunks with gates accumulated in PSUM
    (x-side + h-side matmuls), sigmoid/tanh on the scalar engine with the fused
    per-partition bias, cell updates on the vector engine (c stays fp32, h is
    written bf16 for the next step's matmul),
  - h(len-1) selection is a predicated copy against masks (len == t+1); the
    selected embeddings are transposed token-major, round-tripped through a
    DRAM scratch buffer, and gathered back in the original token order (the
    scratch output buffer is zero-donated, so len-0 paths read back as zero),
  - attention uses a block-diagonal trick: groups of 8 pairs = 80 tokens, all
    4 heads' [80x80] scores packed into one PSUM bank, exp on ACT, block-diag
    mask multiply, column-sum via ones-matmul, reciprocal + partition
    broadcast, and a v.T @ attn matmul giving ctx feature-major directly,
  - attn_out + mean-over-paths + path_proj fold into a single matmul
    (attention rows sum to 1, everything after ctx is linear):
      out = Wf @ sum_k(ctx') + bf,  Wf = wpp @ wao / K,
      bf = wpp @ wao @ bv + wpp @ bao + bpp.
"""

import numpy as np
import ml_dtypes

BF16 = ml_dtypes.bfloat16

NCORES = 8
P, KP, L = 2048, 10, 3
E, H = 256, 512
N_ENT, N_REL = 10000, 200
NHEADS, DH = 4, 128
P_LOC = P // NCORES           # 256 pairs per core
N_LOC = P_LOC * KP            # 2560 paths per core
CH = 512                      # LSTM token chunk
NCH = N_LOC // CH             # 5
NG = CH // 128                # 4 gather groups of 128 per chunk
NTG = N_LOC // 128            # 20 token groups of 128
AG = 80                       # attention group = 8 pairs * 10 paths
PAIRS_G = AG // KP            # 8
AOCH = 320                    # attn chunk (32 pairs)
NAOCH = N_LOC // AOCH         # 8

_PROGS = {}


def _build_program(c_steps):
    import concourse.bass as bass
    import concourse.mybir as mybir
    import concourse.tile as tile
    from concourse import bacc

    f32 = mybir.dt.float32
    bf16 = mybir.dt.bfloat16
    i32 = mybir.dt.int32
    AF = mybir.ActivationFunctionType
    OP = mybir.AluOpType

    NJ = sum(c_steps) * NG            # gather-index columns
    n_proc = sum(1 for s in c_steps if s > 0)

    nc = bacc.Bacc()

    # ---- DRAM parameters (per core) ----
    ent_table = nc.declare_dram_parameter("ent_table_bf", [N_ENT, E], bf16, isOutput=False)
    rel_table = nc.declare_dram_parameter("rel_table_bf", [N_REL, E], bf16, isOutput=False)
    rel_idx_d = nc.declare_dram_parameter("rel_idx_p", [128, NJ], i32, isOutput=False)
    ent_idx_d = nc.declare_dram_parameter("ent_idx_p", [128, NJ], i32, isOutput=False)
    gb_d = nc.declare_dram_parameter("gb_idx", [128, NTG], i32, isOutput=False)
    lens_d = nc.declare_dram_parameter("lens_row", [1, N_LOC], f32, isOutput=False)
    mcat_d = nc.declare_dram_parameter("mcat_t", [2 * E, 4 * H], bf16, isOutput=False)
    whh_d = nc.declare_dram_parameter("whh_t", [H, 4 * H], bf16, isOutput=False)
    gbias_d = nc.declare_dram_parameter("gate_bias", [128, 16], f32, isOutput=False)
    wq_d = nc.declare_dram_parameter("wq_t", [H, H], bf16, isOutput=False)
    wk_d = nc.declare_dram_parameter("wk_t", [H, H], bf16, isOutput=False)
    wv_d = nc.declare_dram_parameter("wv_t", [H, H], bf16, isOutput=False)
    bq_d = nc.declare_dram_parameter("bq_p", [128, 4], f32, isOutput=False)
    bk_d = nc.declare_dram_parameter("bk_p", [128, 4], f32, isOutput=False)
    # fused tail: out = Wf @ sum_k(ctx) + bf, Wf = wpp @ wao / K,
    # bf = wpp @ wao @ bv + wpp @ bao + bpp (attention rows sum to 1)
    wf_d = nc.declare_dram_parameter("wf_t", [H, H], bf16, isOutput=False)
    bf_d = nc.declare_dram_parameter("bf_p", [128, 4], f32, isOutput=False)
    bdm_d = nc.declare_dram_parameter("bd_mask", [128, NHEADS * AG], bf16, isOutput=False)
    # token-major path-embedding scratch (sorted order); declared as an output
    # so PJRT donates a zeroed buffer -> unwritten (len-0 / unprocessed) rows
    # read back as zero
    hscr_d = nc.declare_dram_parameter("h_scr", [N_LOC, H], bf16, isOutput=True)
    out_d = nc.declare_dram_parameter("out", [H, P_LOC], f32, isOutput=True)

    with tile.TileContext(nc) as tc:
        # ---------- persistent pool (spans both phases) ----------
        with tc.tile_pool(name="persist", bufs=1) as pp:
            # h_sel: selected h, feature-major, SORTED token order
            h_sel = [pp.tile([128, n_proc * CH], bf16, name=f"h_sel{i}") for i in range(4)]
            for hs in h_sel:
                nc.vector.memset(hs[:], 0.0)
            # h_org: path_emb, feature-major, ORIGINAL token order
            h_org = [pp.tile([128, N_LOC], bf16, name=f"h_org{i}") for i in range(4)]

            ones_t = pp.tile([128, H], bf16, name="ones_t")
            nc.vector.memset(ones_t[:], 1.0)

            ident = pp.tile([128, 128], bf16, name="ident")
            from concourse.masks import make_identity
            make_identity(nc, ident[:])

            bdm_sb = pp.tile([128, NHEADS * AG], bf16, name="bdm_sb")
            nc.sync.dma_start(out=bdm_sb[:], in_=bdm_d[:, :])

            # ---------- phase 1: encode + LSTM (sorted order) ----------
            with tc.tile_pool(name="lw", bufs=1) as lw, \
                 tc.tile_pool(name="lstm_sb", bufs=2) as ls, \
                 tc.tile_pool(name="gath", bufs=16) as gp, \
                 tc.tile_pool(name="xcat", bufs=8) as xp, \
                 tc.tile_pool(name="sig", bufs=8) as sg, \
                 tc.tile_pool(name="mb", bufs=3) as mbp, \
                 tc.tile_pool(name="xtp", bufs=2, space="PSUM") as xtp, \
                 tc.tile_pool(name="gpsum", bufs=4, space="PSUM") as gpsum:

                mcat_sb = [lw.tile([128, 4 * H], bf16, name=f"mcat{i}") for i in range(4)]
                whh_sb = [lw.tile([128, 4 * H], bf16, name=f"whh{i}") for i in range(4)]
                for i in range(4):
                    nc.sync.dma_start(out=mcat_sb[i][:], in_=mcat_d[i * 128:(i + 1) * 128, :])
                    nc.sync.dma_start(out=whh_sb[i][:], in_=whh_d[i * 128:(i + 1) * 128, :])
                gb_sb = lw.tile([128, NTG], i32, name="gb_sb")
                nc.sync.dma_start(out=gb_sb[:], in_=gb_d[:, :])
                gb_psb = lw.tile([128, 16], f32, name="gb_psb")
                nc.sync.dma_start(out=gb_psb[:], in_=gbias_d[:, :])
                ridx_sb = lw.tile([128, NJ], i32, name="ridx_sb")
                eidx_sb = lw.tile([128, NJ], i32, name="eidx_sb")
                nc.sync.dma_start(out=ridx_sb[:], in_=rel_idx_d[:, :])
                nc.sync.dma_start(out=eidx_sb[:], in_=ent_idx_d[:, :])
                # lens broadcast across partitions once; per-chunk masks via is_equal
                lens_sb = lw.tile([1, N_LOC], f32, name="lens_sb")
                nc.sync.dma_start(out=lens_sb[:], in_=lens_d[:, :])
                lens_b = lw.tile([128, N_LOC], f32, name="lens_b")
                nc.gpsimd.partition_broadcast(lens_b[:], lens_sb[:], channels=128)

                jctr = 0
                wr_insts = []
                for c in range(NCH):
                    h_prev = [None] * 4
                    c_prev = [None] * 4
                    for t in range(c_steps[c]):
                        # gathers (token-major [128, 256] bf16 per 128-token group)
                        gts = []
                        for g in range(NG):
                            j = jctr + g
                            grel = gp.tile([128, E], bf16, name="grel", tag="gath")
                            gent = gp.tile([128, E], bf16, name="gent", tag="gath")
                            nc.gpsimd.indirect_dma_start(
                                out=grel[:], out_offset=None, in_=rel_table[:, :],
                                in_offset=bass.IndirectOffsetOnAxis(
                                    ap=ridx_sb[:, j:j + 1], axis=0))
                            nc.gpsimd.indirect_dma_start(
                                out=gent[:], out_offset=None, in_=ent_table[:, :],
                                in_offset=bass.IndirectOffsetOnAxis(
                                    ap=eidx_sb[:, j:j + 1], axis=0))
                            gts.append((grel, gent))
                        jctr += NG

                        # mask (lens == t+1), [128, CH] from broadcast lens
                        mb = mbp.tile([128, CH], i32, name="mb", tag="mb")
                        nc.vector.tensor_scalar(
                            out=mb[:], in0=lens_b[:, c * CH:(c + 1) * CH],
                            scalar1=float(t + 1), scalar2=None, op0=OP.is_equal)

                        # transpose to feature-major xc [4][128, CH] on PE
                        # (plain matmul against identity; is_transpose=True
                        # would put both sem waits on the LW struct, which
                        # codegen rejects)
                        xt_rel = xtp.tile([128, 2 * CH], f32, name="xt_rel", tag="xt", space="PSUM")
                        xt_ent = xtp.tile([128, 2 * CH], f32, name="xt_ent", tag="xt", space="PSUM")
                        for g in range(NG):
                            grel, gent = gts[g]
                            for half in range(2):
                                nc.tensor.matmul(
                                    out=xt_rel[:, half * CH + g * 128:half * CH + g * 128 + 128],
                                    lhsT=grel[:, half * 128:(half + 1) * 128],
                                    rhs=ident[:], start=True, stop=True)
                                nc.tensor.matmul(
                                    out=xt_ent[:, half * CH + g * 128:half * CH + g * 128 + 128],
                                    lhsT=gent[:, half * 128:(half + 1) * 128],
                                    rhs=ident[:], start=True, stop=True)
                        xc = []
                        for i in range(4):
                            xi = xp.tile([128, CH], bf16, name="xi", tag="xcat")
                            src = (xt_rel, xt_ent)[i // 2]
                            nc.vector.tensor_copy(out=xi[:], in_=src[:, (i % 2) * CH:(i % 2 + 1) * CH])
                            xc.append(xi)

                        # gates: one PSUM bank per gate [i, f, g, o]
                        h_new = [None] * 4
                        c_new = [None] * 4
                        for ft in range(4):
                            gps = {}
                            for gi, m in enumerate((ft, 4 + ft, 8 + ft, 12 + ft)):
                                if t == 0 and gi == 1:
                                    continue  # forget gate unused when c==0
                                gt = gpsum.tile([128, CH], f32, name="gt", tag="gpsum", space="PSUM")
                                gps[gi] = gt
                                for kt in range(4):
                                    nc.tensor.matmul(
                                        out=gt[:], lhsT=mcat_sb[kt][:, m * 128:(m + 1) * 128],
                                        rhs=xc[kt], start=(kt == 0), stop=(t == 0 and kt == 3))
                                if t > 0:
                                    for kt in range(4):
                                        nc.tensor.matmul(
                                            out=gt[:], lhsT=whh_sb[kt][:, m * 128:(m + 1) * 128],
                                            rhs=h_prev[kt], start=False, stop=(kt == 3))
                            si = sg.tile([128, CH], f32, name="si", tag="sig")
                            tg = sg.tile([128, CH], f32, name="tg", tag="sig")
                            so = sg.tile([128, CH], f32, name="so", tag="sig")
                            nc.scalar.activation(out=si[:], in_=gps[0][:],
                                                 func=AF.Sigmoid, bias=gb_psb[:, ft:ft + 1])
                            nc.scalar.activation(out=tg[:], in_=gps[2][:],
                                                 func=AF.Tanh, bias=gb_psb[:, 8 + ft:9 + ft])
                            nc.scalar.activation(out=so[:], in_=gps[3][:],
                                                 func=AF.Sigmoid, bias=gb_psb[:, 12 + ft:13 + ft])
                            cn = ls.tile([128, CH], f32, name="cn", tag=f"c{ft}", bufs=2)
                            if t == 0:
                                nc.vector.tensor_tensor(out=cn[:], in0=si[:], in1=tg[:], op=OP.mult)
                            else:
                                sf = sg.tile([128, CH], f32, name="sf", tag="sig")
                                nc.scalar.activation(out=sf[:], in_=gps[1][:],
                                                     func=AF.Sigmoid, bias=gb_psb[:, 4 + ft:5 + ft])
                                tmp = sg.tile([128, CH], f32, name="tmp", tag="sig")
                                nc.vector.tensor_tensor(out=cn[:], in0=sf[:], in1=c_prev[ft][:], op=OP.mult)
                                nc.vector.tensor_tensor(out=tmp[:], in0=si[:], in1=tg[:], op=OP.mult)
                                nc.vector.tensor_tensor(out=cn[:], in0=cn[:], in1=tmp[:], op=OP.add)
                            tc_t = sg.tile([128, CH], f32, name="tc_t", tag="sig")
                            nc.scalar.activation(out=tc_t[:], in_=cn[:], func=AF.Tanh)
                            hn = ls.tile([128, CH], bf16, name="hn", tag=f"h{ft}", bufs=2)
                            nc.vector.tensor_tensor(out=hn[:], in0=so[:], in1=tc_t[:], op=OP.mult)
                            nc.vector.copy_predicated(
                                out=h_sel[ft][:, c * CH:(c + 1) * CH], mask=mb[:], data=hn[:])
                            h_new[ft] = hn
                            c_new[ft] = cn
                        h_prev = h_new
                        c_prev = c_new

                    # ---- unsort part A: this chunk's h_sel -> token-major
                    # h_scr rows (PE transpose + DMA out), inline so the DMA
                    # overlaps later chunks' LSTM work
                    if c_steps[c] > 0:
                        for g in range(NG):
                            s0 = c * CH + g * 128
                            tp = gpsum.tile([128, CH], f32, name="tp", tag="gpsum", space="PSUM")
                            for ft in range(4):
                                nc.tensor.matmul(
                                    out=tp[:, ft * 128:(ft + 1) * 128],
                                    lhsT=h_sel[ft][:, s0:s0 + 128],
                                    rhs=ident[:], start=True, stop=True)
                            htm = xp.tile([128, CH], bf16, name="htm", tag="xcat")
                            nc.vector.tensor_copy(out=htm[:], in_=tp[:])
                            wr = nc.sync.dma_start(out=hscr_d[s0:s0 + 128, :], in_=htm[:])
                            wr_insts.append(wr)

                # ---- unsort part B: gather back in ORIGINAL token order +
                # transpose feature-major
                for g2 in range(NTG):
                    hg = gp.tile([128, H], bf16, name="hg", tag="gath")
                    rd = nc.gpsimd.indirect_dma_start(
                        out=hg[:], out_offset=None, in_=hscr_d[:, :],
                        in_offset=bass.IndirectOffsetOnAxis(
                            ap=gb_sb[:, g2:g2 + 1], axis=0))
                    for wr in wr_insts:
                        tile.add_dep_helper(rd.ins, wr.ins, reason="h_scr RAW round-trip")
                    tp2 = gpsum.tile([128, CH], f32, name="tp2", tag="gpsum", space="PSUM")
                    for ft in range(4):
                        nc.tensor.matmul(
                            out=tp2[:, ft * 128:(ft + 1) * 128],
                            lhsT=hg[:, ft * 128:(ft + 1) * 128],
                            rhs=ident[:], start=True, stop=True)
                    for ft in range(4):
                        nc.vector.tensor_copy(
                            out=h_org[ft][:, g2 * 128:(g2 + 1) * 128],
                            in_=tp2[:, ft * 128:(ft + 1) * 128])

            # ---------- phase 2: attention (original order, dense) ----------
            NGG = N_LOC // AG  # 32 independent pair-groups
            with tc.tile_pool(name="aw", bufs=1) as aw, \
                 tc.tile_pool(name="asml", bufs=8) as asml, \
                 tc.tile_pool(name="aps2", bufs=2, space="PSUM") as aps2, \
                 tc.tile_pool(name="aps1", bufs=2, space="PSUM") as aps1, \
                 tc.tile_pool(name="aps3", bufs=2, space="PSUM") as aps3, \
                 tc.tile_pool(name="aps4", bufs=2, space="PSUM") as aps4:  # noqa

                wq_sb = [aw.tile([128, H], bf16, name=f"wq{i}") for i in range(4)]
                wk_sb = [aw.tile([128, H], bf16, name=f"wk{i}") for i in range(4)]
                wv_sb = [aw.tile([128, H], bf16, name=f"wv{i}") for i in range(4)]
                wf_sb = [aw.tile([128, H], bf16, name=f"wf{i}") for i in range(4)]
                for i in range(4):
                    nc.sync.dma_start(out=wq_sb[i][:], in_=wq_d[i * 128:(i + 1) * 128, :])
                    nc.sync.dma_start(out=wk_sb[i][:], in_=wk_d[i * 128:(i + 1) * 128, :])
                    nc.sync.dma_start(out=wv_sb[i][:], in_=wv_d[i * 128:(i + 1) * 128, :])
                    nc.sync.dma_start(out=wf_sb[i][:], in_=wf_d[i * 128:(i + 1) * 128, :])
                bq_sb = aw.tile([128, 4], f32, name="bq_sb")
                bk_sb = aw.tile([128, 4], f32, name="bk_sb")
                bf_sb = aw.tile([128, 4], f32, name="bf_sb")
                nc.sync.dma_start(out=bq_sb[:], in_=bq_d[:, :])
                nc.sync.dma_start(out=bk_sb[:], in_=bk_d[:, :])
                nc.sync.dma_start(out=bf_sb[:], in_=bf_d[:, :])

                agg_sb = [aw.tile([128, P_LOC], f32, name=f"agg{i}") for i in range(4)]
                agg_bf = [aw.tile([128, P_LOC], bf16, name=f"aggb{i}") for i in range(4)]

                # A. dense q/k for all tokens (bias added via ACT copy)
                q_all = [aw.tile([128, N_LOC], bf16, name=f"q_all{i}") for i in range(4)]
                k_all = [aw.tile([128, N_LOC], bf16, name=f"k_all{i}") for i in range(4)]
                for m in range(4):
                    for cc in range(NCH):
                        ss = cc * CH
                        qps = aps2.tile([128, CH], f32, name="qps", tag="aps2", space="PSUM")
                        kps = aps2.tile([128, CH], f32, name="kps", tag="aps2", space="PSUM")
                        for kt in range(4):
                            nc.tensor.matmul(
                                out=qps[:], lhsT=wq_sb[kt][:, m * 128:(m + 1) * 128],
                                rhs=h_org[kt][:, ss:ss + CH], start=(kt == 0), stop=(kt == 3))
                            nc.tensor.matmul(
                                out=kps[:], lhsT=wk_sb[kt][:, m * 128:(m + 1) * 128],
                                rhs=h_org[kt][:, ss:ss + CH], start=(kt == 0), stop=(kt == 3))
                        nc.scalar.activation(out=q_all[m][:, ss:ss + CH], in_=qps[:],
                                             func=AF.Identity, bias=bq_sb[:, m:m + 1])
                        nc.scalar.activation(out=k_all[m][:, ss:ss + CH], in_=kps[:],
                                             func=AF.Identity, bias=bk_sb[:, m:m + 1])

                # B. v token-major for all 32 groups (bv folded into bf)
                v_tm = []
                for gg in range(NGG):
                    so_ = gg * AG
                    vp = aps1.tile([128, H], f32, name="vp", tag="aps1", space="PSUM")
                    for kt in range(4):
                        nc.tensor.matmul(
                            out=vp[:AG, :], lhsT=h_org[kt][:, so_:so_ + AG],
                            rhs=wv_sb[kt][:], start=(kt == 0), stop=(kt == 3))
                    vsb = aw.tile([128, H], bf16, name=f"vsb{gg}")
                    nc.scalar.activation(out=vsb[:AG, :], in_=vp[:AG, :], func=AF.Copy)
                    v_tm.append(vsb)

                # C. attention core: 32 independent group chains, processed in
                # batches of 4 so the softmax denominators of 16 (group, head)
                # rows share ONE parallel DVE reciprocal (the [1, N] form is
                # lane-serial and 3x slower)
                GB = 4                      # groups per reciprocal batch
                ctx_all = [aw.tile([128, N_LOC], bf16, name=f"ctx{i}") for i in range(4)]
                for gb in range(NGG // GB):
                    exbs = []
                    den_flat = asml.tile([1, GB * NHEADS * AG], f32, name="den_flat", tag="denf", bufs=2)
                    for gi in range(GB):
                        gg = gb * GB + gi
                        o = gg * AG
                        exb = asml.tile([128, NHEADS * AG], bf16, name="exb", tag="exb")
                        exbs.append(exb)
                        # all 4 heads' scores in one PSUM bank (start only on the
                        # first head: flags=0 writes overwrite untouched columns)
                        sc = aps3.tile([128, NHEADS * AG], f32, name="sc", tag="aps3", space="PSUM")
                        for hh in range(NHEADS):
                            nc.tensor.matmul(
                                out=sc[:AG, hh * AG:(hh + 1) * AG], lhsT=k_all[hh][:, o:o + AG],
                                rhs=q_all[hh][:, o:o + AG], start=(hh == 0), stop=(hh == 3))
                        exu = asml.tile([128, NHEADS * AG], bf16, name="exu", tag="exu")
                        nc.scalar.activation(out=exu[:AG, :], in_=sc[:AG, :],
                                             func=AF.Exp, scale=float(1.0 / np.sqrt(DH)))
                        nc.vector.tensor_tensor(
                            out=exb[:AG, :], in0=exu[:AG, :],
                            in1=bdm_sb[:AG, :], op=OP.mult)
                        cs = aps3.tile([1, NHEADS * AG], f32, name="cs", tag="aps3", space="PSUM")
                        nc.tensor.matmul(out=cs[:1, :], lhsT=ones_t[:AG, :1],
                                         rhs=exb[:AG, :], start=True, stop=True)
                        nc.scalar.activation(
                            out=den_flat[0:1, gi * NHEADS * AG:(gi + 1) * NHEADS * AG],
                            in_=cs[:1, :], func=AF.Copy)
                    # reshape 1x1280 -> 16x80, one parallel reciprocal, reshape back
                    den_p = asml.tile([GB * NHEADS, AG], f32, name="den_p", tag="denp", bufs=2)
                    nc.sync.dma_start(out=den_p[:, :], in_=den_flat[0:1, :])
                    rcp_p = asml.tile([GB * NHEADS, AG], f32, name="rcp_p", tag="rcpp", bufs=2)
                    nc.vector.reciprocal(out=rcp_p[:], in_=den_p[:])
                    rcp_flat = asml.tile([1, GB * NHEADS * AG], f32, name="rcp_flat", tag="rcpf", bufs=2)
                    nc.sync.dma_start(out=rcp_flat[0:1, :], in_=rcp_p[:, :])
                    for gi in range(GB):
                        gg = gb * GB + gi
                        o = gg * AG
                        exb = exbs[gi]
                        rb = asml.tile([128, NHEADS * AG], f32, name="rb", tag="rb", bufs=6)
                        nc.gpsimd.partition_broadcast(
                            rb[:], rcp_flat[0:1, gi * NHEADS * AG:(gi + 1) * NHEADS * AG],
                            channels=128)
                        for hh in range(NHEADS):
                            cxp = aps4.tile([128, AG], f32, name="cxp", tag="aps4", space="PSUM")
                            nc.tensor.matmul(
                                out=cxp[:, :], lhsT=v_tm[gg][:AG, hh * 128:(hh + 1) * 128],
                                rhs=exb[:AG, hh * AG:(hh + 1) * AG], start=True, stop=True)
                            nc.vector.scalar_tensor_tensor(
                                out=ctx_all[hh][:, o:o + AG], in0=cxp[:, :],
                                scalar=1.0, in1=rb[:, hh * AG:(hh + 1) * AG],
                                op0=OP.mult, op1=OP.mult)

                # D. sum ctx over each pair's K paths (tail matmul is folded)
                for m in range(4):
                    nc.vector.reduce_sum(
                        out=agg_sb[m][:],
                        in_=ctx_all[m][:].rearrange("p (a k) -> p a k", k=KP),
                        axis=mybir.AxisListType.X)
                    nc.vector.tensor_copy(out=agg_bf[m][:], in_=agg_sb[m][:])

                # fused attn_out + mean + path_proj: out = Wf @ aggS + bf
                for m in range(4):
                    pps = aps4.tile([128, P_LOC], f32, name="pps", tag="aps4", space="PSUM")
                    for kt in range(4):
                        nc.tensor.matmul(
                            out=pps[:], lhsT=wf_sb[kt][:, m * 128:(m + 1) * 128],
                            rhs=agg_bf[kt][:], start=(kt == 0), stop=(kt == 3))
                    osb = asml.tile([128, P_LOC], f32, name="osb", tag="osb", bufs=4)
                    nc.vector.tensor_scalar_add(out=osb[:], in0=pps[:], scalar1=bf_sb[:, m:m + 1])
                    nc.sync.dma_start(out=out_d[m * 128:(m + 1) * 128, :], in_=osb[:])

    nc.compile()
    return nc


def _chunk_steps(path_lens):
    """Steps needed per 512-token chunk (max over cores), tokens sorted by
    length descending within each core."""
    cs = [0] * NCH
    for core in range(NCORES):
        lens = path_lens[core * P_LOC:(core + 1) * P_LOC].reshape(-1)
        for t in range(L):
            n = int((lens >= t + 1).sum())
            k = -(-n // CH)  # ceil
            for c in range(k):
                cs[c] = max(cs[c], t + 1)
    return tuple(cs)


def _prep_host(inputs, c_steps):
    """Fold weights and lay out indices host-side. Returns (shared, per_core)."""
    f = np.float32
    kg_proj_w = np.asarray(inputs["kg_proj_w"], f)      # [H, E]
    kg_proj_b = np.asarray(inputs["kg_proj_b"], f)      # [H]
    w_ih = np.asarray(inputs["w_ih"], f)                # [4H, 2H]
    w_hh = np.asarray(inputs["w_hh"], f)                # [4H, H]
    b_ih = np.asarray(inputs["b_ih"], f)
    b_hh = np.asarray(inputs["b_hh"], f)
    attn_in_w = np.asarray(inputs["attn_in_w"], f)      # [3H, H]
    attn_in_b = np.asarray(inputs["attn_in_b"], f)
    attn_out_w = np.asarray(inputs["attn_out_w"], f)    # [H, H]
    attn_out_b = np.asarray(inputs["attn_out_b"], f)
    path_proj_w = np.asarray(inputs["path_proj_w"], f)  # [H, H]
    path_proj_b = np.asarray(inputs["path_proj_b"], f)

    W1 = w_ih[:, :H].T                                   # [H, 4H] (rel_p part)
    W2 = w_ih[:, H:].T                                   # [H, 4H] (ent_p part)
    M_r = kg_proj_w.T @ W1                               # [E, 4H]
    M_e = kg_proj_w.T @ W2                               # [E, 4H]
    mcat_t = np.ascontiguousarray(np.concatenate([M_r, M_e], axis=0))  # [2E, 4H]
    gate_bias = kg_proj_b @ W1 + kg_proj_b @ W2 + b_ih + b_hh          # [4H]

    bd = np.zeros((128, AG), f)
    for pg in range(PAIRS_G):
        bd[pg * KP:(pg + 1) * KP, pg * KP:(pg + 1) * KP] = 1.0

    # fused tail: out = Wf @ sum_k(ctx) + bf  (attention rows sum to 1, so bv
    # passes through; mean + attn_out + path_proj are all linear)
    bv = attn_in_b[2 * H:]
    ppw_aow = path_proj_w @ attn_out_w                   # [H, H]
    wf_eff = ppw_aow / KP
    bf_vec = ppw_aow @ bv + path_proj_w @ attn_out_b + path_proj_b

    shared = {
        "ent_table_bf": np.ascontiguousarray(np.asarray(inputs["ent_table"], f).astype(BF16)),
        "rel_table_bf": np.ascontiguousarray(np.asarray(inputs["rel_table"], f).astype(BF16)),
        "mcat_t": np.ascontiguousarray(mcat_t.astype(BF16)),
        "whh_t": np.ascontiguousarray(w_hh.T.astype(BF16)),
        "gate_bias": np.ascontiguousarray(gate_bias.reshape(16, 128).T),
        "wq_t": np.ascontiguousarray(attn_in_w[:H].T.astype(BF16)),
        "wk_t": np.ascontiguousarray(attn_in_w[H:2 * H].T.astype(BF16)),
        "wv_t": np.ascontiguousarray(attn_in_w[2 * H:].T.astype(BF16)),
        "bq_p": np.ascontiguousarray(attn_in_b[:H].reshape(4, 128).T),
        "bk_p": np.ascontiguousarray(attn_in_b[H:2 * H].reshape(4, 128).T),
        "wf_t": np.ascontiguousarray(wf_eff.T.astype(BF16)),
        "bf_p": np.ascontiguousarray(bf_vec.reshape(4, 128).T),
        "bd_mask": np.ascontiguousarray(np.tile(bd, (1, NHEADS)).astype(BF16)),
    }

    rel_idx = np.asarray(inputs["rel_idx"])              # [P, K, L] int32
    ent_idx = np.asarray(inputs["ent_idx"])
    path_lens = np.asarray(inputs["path_lens"])          # [P, K] int32

    NJ = sum(c_steps) * NG
    per_core = []
    for core in range(NCORES):
        sl = slice(core * P_LOC, (core + 1) * P_LOC)
        lens = path_lens[sl].reshape(N_LOC)
        perm = np.argsort(-lens, kind="stable")
        inv = np.empty(N_LOC, np.int64)
        inv[perm] = np.arange(N_LOC)
        ri = rel_idx[sl].reshape(N_LOC, L)[perm]
        ei = ent_idx[sl].reshape(N_LOC, L)[perm]
        rj = np.empty((128, NJ), np.int32)
        ej = np.empty((128, NJ), np.int32)
        jj = 0
        for c in range(NCH):
            for t in range(c_steps[c]):
                for g in range(NG):
                    s0 = c * CH + g * 128
                    rj[:, jj] = ri[s0:s0 + 128, t]
                    ej[:, jj] = ei[s0:s0 + 128, t]
                    jj += 1
        lens_row = lens[perm].reshape(1, N_LOC).astype(f)
        gb = np.ascontiguousarray(inv.reshape(NTG, 128).T.astype(np.int32))
        per_core.append({"rel_idx_p": rj, "ent_idx_p": ej, "lens_row": lens_row,
                         "gb_idx": gb})
    return shared, per_core


def _run(inputs, trace=False):
    from concourse.bass_utils import run_bass_kernel_spmd
    c_steps = _chunk_steps(np.asarray(inputs["path_lens"]))
    if c_steps not in _PROGS:
        _PROGS[c_steps] = _build_program(c_steps)
    prog = _PROGS[c_steps]
    shared, per_core = _prep_host(inputs, c_steps)
    in_maps = [{**shared, **pc} for pc in per_core]
    res = run_bass_kernel_spmd(prog, in_maps, list(range(NCORES)), trace=trace)
    out = np.concatenate([np.ascontiguousarray(r["out"].T) for r in res.results], axis=0)
    return out, res


def kernel(**inputs):
    out, _ = _run(inputs, trace=False)
    return out
